# revision 12
# baseline (speedup 1.0000x reference)
"""Trainium2 Bass kernel for nn_CSA_36971078484033.

Instance-norm over (H,W) per (B,C) with a Dirichlet-weighted prototype affine
(label-conditional bank selection), data-parallel over B on 8 NeuronCores.

  out[b,c,h,w] = (x[b,c,h,w] - mean[b,c]) / sqrt(var[b,c] + eps) * new_std[b,c]
               + new_mean[b,c]
  new_mean = (label==0) ? w@proto_mean_pos : w@proto_mean_neg   (same for std)

Per core: 4 samples = 8 tiles of [128ch, 3136px].  The tiny [64,4]x[64,256]
prototype einsum runs on TensorE with the label selection folded into
host-masked weights (w*(label==0) and w*(label!=0) contribute to pos/neg
banks; the unselected bank's weights are zero).

x and y stream through HBM as bf16 (converted host-side; stats and the affine
math stay f32 on-chip); bf16 round-trip costs ~5e-3 rel err vs the 2e-2 gate.
At bf16 the 12.8 MB/core stream costs ~36us of DMA, so every compute engine
must stay under that.  bn_stats runs at 1x (~4.5us/tile -> 38us: too slow),
so stats come from two accumulate passes instead:
  sum(x):   DVE tensor_scalar copy in 4x bf16 perf mode with accum_out
  sum(x^2): ScalarE Square activation with accum_out
and the affine apply y = x*scl - shf' is a third 4x DVE tensor_scalar.
DVE ~3us/tile and ScalarE ~3us/tile both hide under the ~4.5us/tile DMA.
A 1-tile-deep software pipeline (variance/sqrt/apply of tile i emitted after
the accum passes of tile i+1) keeps the in-order engines from stalling on
cross-engine semaphore waits.
"""

import numpy as np
from contextlib import ExitStack

B, C, H, W = 32, 256, 56, 56
HW = H * W            # 3136
K = 64
EPS = 1e-5
NCORES = 8
BPC = B // NCORES     # 4 samples per core
ROWS = BPC * C        # 1024 DRAM rows per core
PCOLS = 2 * 4 + 4 * 256   # wposT|wnegT|pmp|psp|pmn|psn packed columns

_cache = {}


def _emit(tc, nc, mybir, aps):
    f32 = mybir.dt.float32
    bf16 = mybir.dt.bfloat16
    x_d, packed_d, y_d = aps
    with ExitStack() as ctx:
        consts = ctx.enter_context(tc.tile_pool(name="consts", bufs=1))
        xpool = ctx.enter_context(tc.tile_pool(name="xp", bufs=8))
        ypool = ctx.enter_context(tc.tile_pool(name="yp", bufs=4))
        stats = ctx.enter_context(tc.tile_pool(name="stats", bufs=4))
        psum = ctx.enter_context(tc.tile_pool(name="psum", bufs=2, space="PSUM"))

        # --- tiny inputs packed host-side into ONE [64, 1032] tensor:
        # a single HWDGE push (~0.6us) instead of six (~3.6us) ahead of the
        # first big in-DMA; the byte-bound stream end tracks its start 1:1 ---
        packed_sb = consts.tile([K, PCOLS], f32, tag="packed")
        nc.scalar.dma_start(packed_sb[:], packed_d[:])
        wpos_sb = packed_sb[:, 0:BPC]
        wneg_sb = packed_sb[:, BPC:2 * BPC]
        protos = {}
        for i, name in enumerate(("pmp", "psp", "pmn", "psn")):
            protos[name] = packed_sb[:, 2 * BPC + i * C: 2 * BPC + (i + 1) * C]

        eps_sb = consts.tile([128, 1], f32, tag="eps")
        nc.vector.memset(eps_sb[:], EPS)

        # --- selected new_mean/new_std, channel-major: [128ch, BPC] per half ---
        mean_sel = consts.tile([128, 2 * BPC], f32, tag="mean_sel")
        std_sel = consts.tile([128, 2 * BPC], f32, tag="std_sel")
        for h in range(2):
            cs = slice(h * 128, (h + 1) * 128)
            bs = slice(h * BPC, (h + 1) * BPC)
            pm = psum.tile([128, BPC], f32, tag="ps_mm")
            nc.tensor.matmul(pm[:], protos["pmp"][:, cs], wpos_sb, start=True, stop=False)
            nc.tensor.matmul(pm[:], protos["pmn"][:, cs], wneg_sb, start=False, stop=True)
            nc.vector.tensor_copy(mean_sel[:, bs], pm[:])
            ps = psum.tile([128, BPC], f32, tag="ps_ss")
            nc.tensor.matmul(ps[:], protos["psp"][:, cs], wpos_sb, start=True, stop=False)
            nc.tensor.matmul(ps[:], protos["psn"][:, cs], wneg_sb, start=False, stop=True)
            nc.vector.tensor_copy(std_sel[:, bs], ps[:])

        NT = BPC * 2          # 8 tiles
        Alu = mybir.AluOpType
        N = float(HW)
        K1 = N / (N - 1.0)    # msq_s = (mean*K1)*mean
        K2 = 1.0 / (N - 1.0)  # varu  = sumsq*K2 - msq_s

        # per-engine scratch for the discarded accum-pass outputs (same-engine
        # WAW only, so reuse across tiles costs no semaphores)
        scrpool = ctx.enter_context(tc.tile_pool(name="scr", bufs=1))
        scrA = scrpool.tile([128, HW], bf16, tag="scrA")
        scrS = scrpool.tile([128, HW], bf16, tag="scrS")

        xs, sums, sumsqs, means, msqs = {}, {}, {}, {}, {}

        def accum_phase(ti):
            b, h = divmod(ti, 2)
            r0 = b * C + h * 128
            x_sb = xpool.tile([128, HW], bf16, tag="xt")
            nc.sync.dma_start(x_sb[:], x_d[r0:r0 + 128, :])
            xs[ti] = x_sb
            # sum(x): 4x-perf-mode copy with accumulate (out discarded)
            s = stats.tile([128, 1], f32, tag="sum")
            nc.vector.tensor_scalar(scrA[:], x_sb[:], 1.0, 0.0,
                                    Alu.mult, Alu.add, accum_out=s[:])
            sums[ti] = s
            # sum(x^2): ScalarE Square with accumulate (out discarded)
            sq = stats.tile([128, 1], f32, tag="sumsq")
            nc.scalar.activation(scrS[:], x_sb[:],
                                 mybir.ActivationFunctionType.Square,
                                 accum_out=sq[:])
            sumsqs[ti] = sq
            m = stats.tile([128, 1], f32, tag="mean")
            nc.vector.tensor_scalar_mul(m[:], s[:], 1.0 / N)
            means[ti] = m
            q = stats.tile([128, 1], f32, tag="msq")
            nc.vector.scalar_tensor_tensor(q[:], m[:], K1, m[:],
                                           Alu.mult, Alu.mult)
            msqs[ti] = q

        def finish_phase(ti):
            b, h = divmod(ti, 2)
            r0 = b * C + h * 128
            col = h * BPC + b
            varu = stats.tile([128, 1], f32, tag="varu")
            nc.vector.scalar_tensor_tensor(varu[:], sumsqs[ti][:], K2,
                                           msqs[ti][:], Alu.mult, Alu.subtract)
            stdv = stats.tile([128, 1], f32, tag="stdv")
            nc.scalar.activation(stdv[:], varu[:],
                                 mybir.ActivationFunctionType.Sqrt,
                                 bias=eps_sb[:])
            rstd = stats.tile([128, 1], f32, tag="rstd")
            nc.vector.reciprocal(rstd[:], stdv[:])
            scl = stats.tile([128, 1], f32, tag="scl")
            nc.vector.tensor_mul(scl[:], rstd[:], std_sel[:, col:col + 1])
            # shf' = mean*scl - mean_sel, applied as y = x*scl - shf'
            shf = stats.tile([128, 1], f32, tag="shf")
            nc.vector.scalar_tensor_tensor(shf[:], means[ti][:], scl[:],
                                           mean_sel[:, col:col + 1],
                                           Alu.mult, Alu.subtract)
            # apply: third 4x-mode DVE pass
            y_sb = ypool.tile([128, HW], bf16, tag="yt")
            nc.vector.tensor_scalar(y_sb[:], xs[ti][:], scl[:], shf[:],
                                    Alu.mult, Alu.subtract)
            # out-DMAs ride the Activation HWDGE ring: the Sync ring is FIFO,
            # so an out waiting on compute would head-of-line block later ins
            nc.scalar.dma_start(y_d[r0:r0 + 128, :], y_sb[:])

        # 1-deep software pipeline: tile i's variance/sqrt/apply emits after
        # tile i+1's accumulate passes, so in-order engines never stall on a
        # cross-engine result that isn't ready yet
        for ti in range(NT):
            accum_phase(ti)
            if ti >= 1:
                finish_phase(ti - 1)
        finish_phase(NT - 1)


def _program():
    if "nc" in _cache:
        return _cache["nc"]
    import concourse.bass as bass  # noqa: F401
    import concourse.tile as tile
    from concourse import bacc, mybir

    f32 = mybir.dt.float32
    bf16 = mybir.dt.bfloat16
    nc = bacc.Bacc("TRN2", target_bir_lowering=False, debug=False,
                   num_devices=NCORES)
    aps = [
        nc.dram_tensor("x", [ROWS, HW], bf16, kind="ExternalInput").ap(),
        nc.dram_tensor("packed", [K, PCOLS], f32, kind="ExternalInput").ap(),
        nc.dram_tensor("y", [ROWS, HW], bf16, kind="ExternalOutput").ap(),
    ]
    with tile.TileContext(nc) as tc:
        _emit(tc, nc, mybir, aps)
    nc.compile()
    _cache["nc"] = nc
    return nc


def _run(inputs, trace=False, trace_cores=None):
    import ml_dtypes
    from concourse import bass_utils

    nc = _program()

    x = np.asarray(inputs["x"], dtype=np.float32)
    label = np.asarray(inputs["label"])
    w = np.asarray(inputs["combine_weights"], dtype=np.float32)
    pmp = np.ascontiguousarray(np.asarray(inputs["proto_mean_pos"], dtype=np.float32))
    psp = np.ascontiguousarray(np.asarray(inputs["proto_std_pos"], dtype=np.float32))
    pmn = np.ascontiguousarray(np.asarray(inputs["proto_mean_neg"], dtype=np.float32))
    psn = np.ascontiguousarray(np.asarray(inputs["proto_std_neg"], dtype=np.float32))

    is_pos = (label == 0).astype(np.float32)[:, None]   # [B,1]
    wpos = w * is_pos                                   # [B,K]
    wneg = w * (1.0 - is_pos)

    x_bf = x.reshape(NCORES, ROWS, HW).astype(ml_dtypes.bfloat16)
    in_maps = []
    for c in range(NCORES):
        bs = slice(c * BPC, (c + 1) * BPC)
        packed = np.concatenate(
            [wpos[bs].T, wneg[bs].T, pmp, psp, pmn, psn], axis=1)
        in_maps.append({
            "x": np.ascontiguousarray(x_bf[c]),
            "packed": np.ascontiguousarray(packed),
        })

    res = bass_utils.run_bass_kernel_spmd(
        nc, in_maps, core_ids=list(range(NCORES)),
        trace=trace, trace_cores=trace_cores,
    )
    out = np.concatenate(
        [np.asarray(res.results[c]["y"], dtype=np.float32).reshape(BPC, C, H, W)
         for c in range(NCORES)],
        axis=0,
    )
    return out, res


def kernel(**inputs):
    out, _ = _run(inputs, trace=False)
    return out



# revision 14
# speedup vs baseline: 1.1710x; 1.1710x over previous
"""Trainium2 Bass kernel for nn_CSA_36971078484033.

Instance-norm over (H,W) per (B,C) with a Dirichlet-weighted prototype affine
(label-conditional bank selection), data-parallel over B on 8 NeuronCores.

  out[b,c,h,w] = (x[b,c,h,w] - mean[b,c]) / sqrt(var[b,c] + eps) * new_std[b,c]
               + new_mean[b,c]
  new_mean = (label==0) ? w@proto_mean_pos : w@proto_mean_neg   (same for std)

Per core: 4 samples = 8 tiles of [128ch, 3136px].  The tiny [64,4]x[64,256]
prototype einsum runs on TensorE with the label selection folded into
host-masked weights (w*(label==0) and w*(label!=0) contribute to pos/neg
banks; the unselected bank's weights are zero).

x and y stream through HBM as bf16 (converted host-side; stats and the affine
math stay f32 on-chip); bf16 round-trip costs ~5e-3 rel err vs the 2e-2 gate.
At bf16 the 12.8 MB/core stream costs ~36us of DMA, so every compute engine
must stay under that.  bn_stats runs at 1x (~4.5us/tile -> 38us: too slow),
so stats come from two accumulate passes instead:
  sum(x):   DVE tensor_scalar copy in 4x bf16 perf mode with accum_out
  sum(x^2): ScalarE Square activation with accum_out
and the affine apply y = x*scl - shf' is a third 4x DVE tensor_scalar.
DVE ~3us/tile and ScalarE ~3us/tile both hide under the ~4.5us/tile DMA.
A 1-tile-deep software pipeline (variance/sqrt/apply of tile i emitted after
the accum passes of tile i+1) keeps the in-order engines from stalling on
cross-engine semaphore waits.
"""

import numpy as np
from contextlib import ExitStack

B, C, H, W = 32, 256, 56, 56
HW = H * W            # 3136
K = 64
EPS = 1e-5
NCORES = 8
BPC = B // NCORES     # 4 samples per core
ROWS = BPC * C        # 1024 DRAM rows per core
PCOLS = 2 * 4 + 4 * 256   # wposT|wnegT|pmp|psp|pmn|psn packed columns

_cache = {}


def _emit(tc, nc, mybir, aps):
    f32 = mybir.dt.float32
    bf16 = mybir.dt.bfloat16
    x_d, packed_d, y_d = aps
    with ExitStack() as ctx:
        consts = ctx.enter_context(tc.tile_pool(name="consts", bufs=1))
        xpool = ctx.enter_context(tc.tile_pool(name="xp", bufs=8))
        ypool = ctx.enter_context(tc.tile_pool(name="yp", bufs=4))
        stats = ctx.enter_context(tc.tile_pool(name="stats", bufs=4))
        psum = ctx.enter_context(tc.tile_pool(name="psum", bufs=2, space="PSUM"))

        # --- tiny inputs packed host-side into ONE [64, 1032] tensor:
        # a single HWDGE push (~0.6us) instead of six (~3.6us) ahead of the
        # first big in-DMA; the byte-bound stream end tracks its start 1:1 ---
        packed_sb = consts.tile([K, PCOLS], f32, tag="packed")
        nc.scalar.dma_start(packed_sb[:], packed_d[:])
        wpos_sb = packed_sb[:, 0:BPC]
        wneg_sb = packed_sb[:, BPC:2 * BPC]
        protos = {}
        for i, name in enumerate(("pmp", "psp", "pmn", "psn")):
            protos[name] = packed_sb[:, 2 * BPC + i * C: 2 * BPC + (i + 1) * C]

        eps_sb = consts.tile([128, 1], f32, tag="eps")
        nc.vector.memset(eps_sb[:], EPS)

        # --- selected new_mean/new_std, channel-major: [128ch, BPC] per half ---
        mean_sel = consts.tile([128, 2 * BPC], f32, tag="mean_sel")
        std_sel = consts.tile([128, 2 * BPC], f32, tag="std_sel")
        for h in range(2):
            cs = slice(h * 128, (h + 1) * 128)
            bs = slice(h * BPC, (h + 1) * BPC)
            pm = psum.tile([128, BPC], f32, tag="ps_mm")
            nc.tensor.matmul(pm[:], protos["pmp"][:, cs], wpos_sb, start=True, stop=False)
            nc.tensor.matmul(pm[:], protos["pmn"][:, cs], wneg_sb, start=False, stop=True)
            nc.vector.tensor_copy(mean_sel[:, bs], pm[:])
            ps = psum.tile([128, BPC], f32, tag="ps_ss")
            nc.tensor.matmul(ps[:], protos["psp"][:, cs], wpos_sb, start=True, stop=False)
            nc.tensor.matmul(ps[:], protos["psn"][:, cs], wneg_sb, start=False, stop=True)
            nc.vector.tensor_copy(std_sel[:, bs], ps[:])

        NT = BPC * 2          # 8 tiles
        GT = 2                # tiles per group (batched scalar chain)
        NG = NT // GT
        Alu = mybir.AluOpType
        AF = mybir.ActivationFunctionType
        N = float(HW)
        K1 = N / (N - 1.0)    # msq_s = (mean*K1)*mean
        K2 = 1.0 / (N - 1.0)  # varu  = sumsq*K2 - msq_s

        # engine-local scratches, reused across tiles (same-engine WAW only,
        # so no semaphores): ScalarE square output + DVE adder-tree levels
        scrpool = ctx.enter_context(tc.tile_pool(name="scr", bufs=1))
        scrS = scrpool.tile([128, HW], bf16, tag="scrS")
        h1 = scrpool.tile([128, HW // 2], bf16, tag="h1")
        h2 = scrpool.tile([128, HW // 4], bf16, tag="h2")
        h3 = scrpool.tile([128, HW // 8], bf16, tag="h3")

        # all in-DMAs up front: the Sync ring does nothing else, so the
        # in-stream runs at full rate from the start
        xs = []
        for ti in range(NT):
            b, h = divmod(ti, 2)
            r0 = b * C + h * 128
            x_sb = xpool.tile([128, HW], bf16, tag="xt")
            nc.sync.dma_start(x_sb[:], x_d[r0:r0 + 128, :])
            xs.append(x_sb)

        sums_g, sumsqs_g, scls_g, shfs_g = {}, {}, {}, {}

        def accum_group(g):
            # per tile: sum(x) via a bf16 pairwise adder tree (packed 2-byte
            # tensor_tensor adds run in the DVE 2x perf mode; a direct
            # tensor_reduce or accum_out pass runs 1x = 3.4us) and sum(x^2)
            # via ScalarE Square-with-accumulate
            s = stats.tile([128, GT], f32, tag="sum")
            sq = stats.tile([128, GT], f32, tag="sumsq")
            for h in range(GT):
                x_sb = xs[g * GT + h]
                nc.vector.tensor_add(h1[:], x_sb[:, :HW // 2], x_sb[:, HW // 2:])
                nc.vector.tensor_add(h2[:], h1[:, :HW // 4], h1[:, HW // 4:])
                nc.vector.tensor_add(h3[:], h2[:, :HW // 8], h2[:, HW // 8:])
                nc.vector.tensor_reduce(s[:, h:h + 1], h3[:],
                                        axis=mybir.AxisListType.X, op=Alu.add)
                nc.scalar.activation(scrS[:], x_sb[:], AF.Square,
                                     accum_out=sq[:, h:h + 1])
            sums_g[g] = s
            sumsqs_g[g] = sq

        def finish_group(g):
            # batched scalar chain on [128, GT] columns; sel tables are laid
            # out [128, 2*BPC] with col = h*BPC + b, so group g (b=g, h=0..1)
            # gathers the strided columns g::BPC
            sel = slice(g, None, BPC)
            mean = stats.tile([128, GT], f32, tag="mean")
            nc.vector.tensor_scalar_mul(mean[:], sums_g[g][:], 1.0 / N)
            msq = stats.tile([128, GT], f32, tag="msq")
            nc.vector.scalar_tensor_tensor(msq[:], mean[:], K1, mean[:],
                                           Alu.mult, Alu.mult)
            varu = stats.tile([128, GT], f32, tag="varu")
            nc.vector.scalar_tensor_tensor(varu[:], sumsqs_g[g][:], K2,
                                           msq[:], Alu.mult, Alu.subtract)
            stdv = stats.tile([128, GT], f32, tag="stdv")
            nc.scalar.activation(stdv[:], varu[:], AF.Sqrt, bias=eps_sb[:])
            rstd = stats.tile([128, GT], f32, tag="rstd")
            nc.vector.reciprocal(rstd[:], stdv[:])
            scl = stats.tile([128, GT], f32, tag="scl")
            nc.vector.tensor_mul(scl[:], rstd[:], std_sel[:, sel])
            # shf' = mean*scl - mean_sel, applied below as y = x*scl - shf'
            tmp = stats.tile([128, GT], f32, tag="tmp")
            nc.vector.tensor_mul(tmp[:], mean[:], scl[:])
            shf = stats.tile([128, GT], f32, tag="shf")
            nc.vector.tensor_sub(shf[:], tmp[:], mean_sel[:, sel])
            scls_g[g], shfs_g[g] = scl, shf
            for h in range(GT):
                ti = g * GT + h
                b, hh = divmod(ti, 2)
                r0 = b * C + hh * 128
                # apply: 4x-perf-mode DVE tensor_scalar
                y_sb = ypool.tile([128, HW], bf16, tag="yt")
                nc.vector.tensor_scalar(y_sb[:], xs[ti][:],
                                        scl[:, h:h + 1], shf[:, h:h + 1],
                                        Alu.mult, Alu.subtract)
                # out-DMAs ride the Activation HWDGE ring (Sync's FIFO would
                # order them behind still-streaming ins)
                nc.scalar.dma_start(y_d[r0:r0 + 128, :], y_sb[:])

        # 1-group-deep software pipeline: group g's chain+apply emits after
        # group g+1's accumulate passes, so the in-order engines don't stall
        # on cross-engine results that aren't ready yet
        for g in range(NG):
            accum_group(g)
            if g >= 1:
                finish_group(g - 1)
        finish_group(NG - 1)


def _program():
    if "nc" in _cache:
        return _cache["nc"]
    import concourse.bass as bass  # noqa: F401
    import concourse.tile as tile
    from concourse import bacc, mybir

    f32 = mybir.dt.float32
    bf16 = mybir.dt.bfloat16
    nc = bacc.Bacc("TRN2", target_bir_lowering=False, debug=False,
                   num_devices=NCORES)
    aps = [
        nc.dram_tensor("x", [ROWS, HW], bf16, kind="ExternalInput").ap(),
        nc.dram_tensor("packed", [K, PCOLS], f32, kind="ExternalInput").ap(),
        nc.dram_tensor("y", [ROWS, HW], bf16, kind="ExternalOutput").ap(),
    ]
    with tile.TileContext(nc) as tc:
        _emit(tc, nc, mybir, aps)
    nc.compile()
    _cache["nc"] = nc
    return nc


def _run(inputs, trace=False, trace_cores=None):
    import ml_dtypes
    from concourse import bass_utils

    nc = _program()

    x = np.asarray(inputs["x"], dtype=np.float32)
    label = np.asarray(inputs["label"])
    w = np.asarray(inputs["combine_weights"], dtype=np.float32)
    pmp = np.ascontiguousarray(np.asarray(inputs["proto_mean_pos"], dtype=np.float32))
    psp = np.ascontiguousarray(np.asarray(inputs["proto_std_pos"], dtype=np.float32))
    pmn = np.ascontiguousarray(np.asarray(inputs["proto_mean_neg"], dtype=np.float32))
    psn = np.ascontiguousarray(np.asarray(inputs["proto_std_neg"], dtype=np.float32))

    is_pos = (label == 0).astype(np.float32)[:, None]   # [B,1]
    wpos = w * is_pos                                   # [B,K]
    wneg = w * (1.0 - is_pos)

    x_bf = x.reshape(NCORES, ROWS, HW).astype(ml_dtypes.bfloat16)
    in_maps = []
    for c in range(NCORES):
        bs = slice(c * BPC, (c + 1) * BPC)
        packed = np.concatenate(
            [wpos[bs].T, wneg[bs].T, pmp, psp, pmn, psn], axis=1)
        in_maps.append({
            "x": np.ascontiguousarray(x_bf[c]),
            "packed": np.ascontiguousarray(packed),
        })

    res = bass_utils.run_bass_kernel_spmd(
        nc, in_maps, core_ids=list(range(NCORES)),
        trace=trace, trace_cores=trace_cores,
    )
    out = np.concatenate(
        [np.asarray(res.results[c]["y"], dtype=np.float32).reshape(BPC, C, H, W)
         for c in range(NCORES)],
        axis=0,
    )
    return out, res


def kernel(**inputs):
    out, _ = _run(inputs, trace=False)
    return out



# revision 19
# speedup vs baseline: 1.2652x; 1.0805x over previous
"""Trainium2 Bass kernel for nn_CSA_36971078484033.

Instance-norm over (H,W) per (B,C) with a Dirichlet-weighted prototype affine
(label-conditional bank selection), data-parallel over B on 8 NeuronCores.

  out[b,c,h,w] = (x[b,c,h,w] - mean[b,c]) / sqrt(var[b,c] + eps) * new_std[b,c]
               + new_mean[b,c]
  new_mean = (label==0) ? w@proto_mean_pos : w@proto_mean_neg   (same for std)

Per core: 4 samples = 8 tiles of [128ch, 3136px].  The tiny [64,4]x[64,256]
prototype einsum runs on TensorE with the label selection folded into
host-masked weights (w*(label==0) and w*(label!=0) contribute to pos/neg
banks; the unselected bank's weights are zero).

x and y stream through HBM as bf16 (converted host-side; stats and the affine
math stay f32 on-chip); bf16 round-trip costs ~5e-3 rel err vs the 2e-2 gate.
At bf16 the 12.8 MB/core stream costs ~36us of DMA, so every compute engine
must stay under that.  bn_stats runs at 1x (~4.5us/tile -> 38us: too slow),
so stats come from two accumulate passes instead:
  sum(x):   DVE tensor_scalar copy in 4x bf16 perf mode with accum_out
  sum(x^2): ScalarE Square activation with accum_out
and the affine apply y = x*scl - shf' is a third 4x DVE tensor_scalar.
DVE ~3us/tile and ScalarE ~3us/tile both hide under the ~4.5us/tile DMA.
A 1-tile-deep software pipeline (variance/sqrt/apply of tile i emitted after
the accum passes of tile i+1) keeps the in-order engines from stalling on
cross-engine semaphore waits.
"""

import numpy as np
from contextlib import ExitStack

B, C, H, W = 32, 256, 56, 56
HW = H * W            # 3136
K = 64
EPS = 1e-5
NCORES = 8
BPC = B // NCORES     # 4 samples per core
ROWS = BPC * C        # 1024 DRAM rows per core
PCOLS = 2 * 4 + 4 * 256   # wposT|wnegT|pmp|psp|pmn|psn packed columns

_cache = {}


def _emit(tc, nc, mybir, aps):
    f32 = mybir.dt.float32
    bf16 = mybir.dt.bfloat16
    x_d, packed_d, y_d = aps
    with ExitStack() as ctx:
        consts = ctx.enter_context(tc.tile_pool(name="consts", bufs=1))
        xpool = ctx.enter_context(tc.tile_pool(name="xp", bufs=8))
        ypool = ctx.enter_context(tc.tile_pool(name="yp", bufs=4))
        stats = ctx.enter_context(tc.tile_pool(name="stats", bufs=4))
        psum = ctx.enter_context(tc.tile_pool(name="psum", bufs=2, space="PSUM"))

        # --- tiny inputs packed host-side into ONE [64, 1032] tensor:
        # a single HWDGE push (~0.6us) instead of six (~3.6us); on the Sync
        # ring ahead of the x stream so ScalarE never touches a DMA ---
        packed_sb = consts.tile([K, PCOLS], f32, tag="packed")
        nc.sync.dma_start(packed_sb[:], packed_d[:])
        wpos_sb = packed_sb[:, 0:BPC]
        wneg_sb = packed_sb[:, BPC:2 * BPC]
        protos = {}
        for i, name in enumerate(("pmp", "psp", "pmn", "psn")):
            protos[name] = packed_sb[:, 2 * BPC + i * C: 2 * BPC + (i + 1) * C]

        eps_sb = consts.tile([128, 1], f32, tag="eps")
        nc.gpsimd.memset(eps_sb[:], EPS)

        # --- selected new_mean/new_std, channel-major: [128ch, BPC] per half.
        # Matmuls emit now (TensorE is free); the PSUM->SBUF copies are
        # deferred into the group loop so DVE's in-order stream starts on the
        # first adder tree instead of stalling behind the matmul prologue ---
        mean_sel = consts.tile([128, 2 * BPC], f32, tag="mean_sel")
        std_sel = consts.tile([128, 2 * BPC], f32, tag="std_sel")
        sel_copies = []
        for h in range(2):
            cs = slice(h * 128, (h + 1) * 128)
            bs = slice(h * BPC, (h + 1) * BPC)
            pm = psum.tile([128, BPC], f32, tag="ps_mm")
            nc.tensor.matmul(pm[:], protos["pmp"][:, cs], wpos_sb, start=True, stop=False)
            nc.tensor.matmul(pm[:], protos["pmn"][:, cs], wneg_sb, start=False, stop=True)
            sel_copies.append((mean_sel[:, bs], pm))
            ps = psum.tile([128, BPC], f32, tag="ps_ss")
            nc.tensor.matmul(ps[:], protos["psp"][:, cs], wpos_sb, start=True, stop=False)
            nc.tensor.matmul(ps[:], protos["psn"][:, cs], wneg_sb, start=False, stop=True)
            sel_copies.append((std_sel[:, bs], ps))

        NT = BPC * 2          # 8 tiles
        GT = 2                # tiles per group (batched scalar chain)
        NG = NT // GT
        Alu = mybir.AluOpType
        AF = mybir.ActivationFunctionType
        N = float(HW)
        K1 = N / (N - 1.0)    # msq_s = (mean*K1)*mean
        K2 = 1.0 / (N - 1.0)  # varu  = sumsq*K2 - msq_s

        # engine-local scratches, reused across tiles (same-engine WAW only,
        # so no semaphores): ScalarE square output + DVE adder-tree levels
        scrpool = ctx.enter_context(tc.tile_pool(name="scr", bufs=1))
        scrS = scrpool.tile([128, HW], bf16, tag="scrS")
        h1 = scrpool.tile([128, HW // 2], bf16, tag="h1")
        h2 = scrpool.tile([128, HW // 4], bf16, tag="h2")
        h3 = scrpool.tile([128, HW // 8], bf16, tag="h3")

        # all in-DMAs up front: the Sync ring does nothing else, so the
        # in-stream runs at full rate from the start
        xs = []
        for ti in range(NT):
            b, h = divmod(ti, 2)
            r0 = b * C + h * 128
            x_sb = xpool.tile([128, HW], bf16, tag="xt")
            nc.sync.dma_start(x_sb[:], x_d[r0:r0 + 128, :])
            xs.append(x_sb)

        sums_g, sumsqs_g, scls_g, shfs_g = {}, {}, {}, {}

        def accum_group(g):
            # per tile: sum(x) via a bf16 pairwise adder tree (packed 2-byte
            # tensor_tensor adds run in the DVE 2x perf mode; a direct
            # tensor_reduce or accum_out pass runs 1x = 3.4us) and sum(x^2)
            # via ScalarE Square-with-accumulate
            s = stats.tile([128, GT], f32, tag="sum")
            sq = stats.tile([128, GT], f32, tag="sumsq")
            for h in range(GT):
                x_sb = xs[g * GT + h]
                nc.vector.tensor_add(h1[:], x_sb[:, :HW // 2], x_sb[:, HW // 2:])
                nc.vector.tensor_add(h2[:], h1[:, :HW // 4], h1[:, HW // 4:])
                nc.vector.tensor_add(h3[:], h2[:, :HW // 8], h2[:, HW // 8:])
                nc.vector.tensor_reduce(s[:, h:h + 1], h3[:],
                                        axis=mybir.AxisListType.X, op=Alu.add)
                nc.scalar.activation(scrS[:], x_sb[:], AF.Square,
                                     accum_out=sq[:, h:h + 1])
            sums_g[g] = s
            sumsqs_g[g] = sq

        def finish_group(g):
            # batched scalar chain on [128, GT] columns; sel tables are laid
            # out [128, 2*BPC] with col = h*BPC + b, so group g (b=g, h=0..1)
            # gathers the strided columns g::BPC
            sel = slice(g, None, BPC)
            mean = stats.tile([128, GT], f32, tag="mean")
            nc.vector.tensor_scalar_mul(mean[:], sums_g[g][:], 1.0 / N)
            msq = stats.tile([128, GT], f32, tag="msq")
            nc.vector.scalar_tensor_tensor(msq[:], mean[:], K1, mean[:],
                                           Alu.mult, Alu.mult)
            varu = stats.tile([128, GT], f32, tag="varu")
            nc.vector.scalar_tensor_tensor(varu[:], sumsqs_g[g][:], K2,
                                           msq[:], Alu.mult, Alu.subtract)
            stdv = stats.tile([128, GT], f32, tag="stdv")
            nc.scalar.activation(stdv[:], varu[:], AF.Sqrt, bias=eps_sb[:])
            rstd = stats.tile([128, GT], f32, tag="rstd")
            nc.vector.reciprocal(rstd[:], stdv[:])
            scl = stats.tile([128, GT], f32, tag="scl")
            nc.vector.tensor_mul(scl[:], rstd[:], std_sel[:, sel])
            # shf' = mean*scl - mean_sel, applied below as y = x*scl - shf'
            tmp = stats.tile([128, GT], f32, tag="tmp")
            nc.vector.tensor_mul(tmp[:], mean[:], scl[:])
            shf = stats.tile([128, GT], f32, tag="shf")
            nc.vector.tensor_sub(shf[:], tmp[:], mean_sel[:, sel])
            scls_g[g], shfs_g[g] = scl, shf
            for h in range(GT):
                ti = g * GT + h
                b, hh = divmod(ti, 2)
                r0 = b * C + hh * 128
                # apply: 4x-perf-mode DVE tensor_scalar
                y_sb = ypool.tile([128, HW], bf16, tag="yt")
                nc.vector.tensor_scalar(y_sb[:], xs[ti][:],
                                        scl[:, h:h + 1], shf[:, h:h + 1],
                                        Alu.mult, Alu.subtract)
                # out-DMAs ride GpSimd's SWDGE: a dispatch costs ~0.6us of
                # engine time AND blocks the in-order engine on the apply's
                # completion, which ScalarE (the Square backbone) can't absorb
                nc.gpsimd.dma_start(y_d[r0:r0 + 128, :], y_sb[:])

        # 1-group-deep software pipeline: group g's chain+apply emits after
        # group g+1's accumulate passes, so the in-order engines don't stall
        # on cross-engine results that aren't ready yet
        for g in range(NG):
            accum_group(g)
            if g == 1:
                # matmuls finished long ago; copies run with no DVE stall
                for dst, src in sel_copies:
                    nc.vector.tensor_copy(dst, src[:])
            if g >= 1:
                finish_group(g - 1)
        finish_group(NG - 1)


def _program():
    if "nc" in _cache:
        return _cache["nc"]
    import concourse.bass as bass  # noqa: F401
    import concourse.tile as tile
    from concourse import bacc, mybir

    f32 = mybir.dt.float32
    bf16 = mybir.dt.bfloat16
    nc = bacc.Bacc("TRN2", target_bir_lowering=False, debug=False,
                   num_devices=NCORES)
    aps = [
        nc.dram_tensor("x", [ROWS, HW], bf16, kind="ExternalInput").ap(),
        nc.dram_tensor("packed", [K, PCOLS], f32, kind="ExternalInput").ap(),
        nc.dram_tensor("y", [ROWS, HW], bf16, kind="ExternalOutput").ap(),
    ]
    with tile.TileContext(nc) as tc:
        _emit(tc, nc, mybir, aps)
    nc.compile()
    _cache["nc"] = nc
    return nc


def _run(inputs, trace=False, trace_cores=None):
    import ml_dtypes
    from concourse import bass_utils

    nc = _program()

    x = np.asarray(inputs["x"], dtype=np.float32)
    label = np.asarray(inputs["label"])
    w = np.asarray(inputs["combine_weights"], dtype=np.float32)
    pmp = np.ascontiguousarray(np.asarray(inputs["proto_mean_pos"], dtype=np.float32))
    psp = np.ascontiguousarray(np.asarray(inputs["proto_std_pos"], dtype=np.float32))
    pmn = np.ascontiguousarray(np.asarray(inputs["proto_mean_neg"], dtype=np.float32))
    psn = np.ascontiguousarray(np.asarray(inputs["proto_std_neg"], dtype=np.float32))

    is_pos = (label == 0).astype(np.float32)[:, None]   # [B,1]
    wpos = w * is_pos                                   # [B,K]
    wneg = w * (1.0 - is_pos)

    x_bf = x.reshape(NCORES, ROWS, HW).astype(ml_dtypes.bfloat16)
    in_maps = []
    for c in range(NCORES):
        bs = slice(c * BPC, (c + 1) * BPC)
        packed = np.concatenate(
            [wpos[bs].T, wneg[bs].T, pmp, psp, pmn, psn], axis=1)
        in_maps.append({
            "x": np.ascontiguousarray(x_bf[c]),
            "packed": np.ascontiguousarray(packed),
        })

    res = bass_utils.run_bass_kernel_spmd(
        nc, in_maps, core_ids=list(range(NCORES)),
        trace=trace, trace_cores=trace_cores,
    )
    out = np.concatenate(
        [np.asarray(res.results[c]["y"], dtype=np.float32).reshape(BPC, C, H, W)
         for c in range(NCORES)],
        axis=0,
    )
    return out, res


def kernel(**inputs):
    out, _ = _run(inputs, trace=False)
    return out

